# revision 2
# baseline (speedup 1.0000x reference)
"""KV-cache scatter kernel for Trainium2 (8 NeuronCores, batch-sharded).

Problem: k_out = k_cache.at[b, :, input_pos[b, t], :].set(k[b, :, t, :])
         (same for v). Shapes: k/v (B,H,T,D)=(8,16,16,128),
         caches (B,H,S,D)=(8,16,4096,128), input_pos (B,T).

Strategy: shard the batch dim across the 8 cores (one batch row each),
and update the caches IN PLACE instead of copying them. The caches are
passed to the device as donated output-init buffers (the jax/PJRT
donation path reuses the staged input buffer as the NEFF output buffer,
so output elements the program never writes retain the cache contents).
The device program is then only the scatter of the update rows.

Two device programs:
- "wide": when each batch row's positions are a contiguous run of T
  aligned to T (the arange case), the cache is viewed as [H*S/T, T*D]
  and each core scatters 16 rows of 8 KiB per cache — one indirect DMA
  per cache, sourced from a single packed SBUF blob.
- "narrow": generic fallback for arbitrary positions — 256 rows of
  512 B per cache via 2+2 indirect DMAs (the baseline scatter scheme).
"""

import numpy as np

B, H, T, D = 8, 16, 16, 128
S = 4096
HS = H * S            # rows in the flattened (H*S, D) cache view
NROW = H * T          # 256 narrow update rows per batch element
P = 128               # SBUF partitions
WR = H * S // T       # 4096 rows in the wide (WR, T*D) cache view
WC = T * D            # 2048 elements per wide row (8 KiB)

_PROGRAMS = {}        # (mode, n_iters) -> bass program
_JITTED = {}          # id(nc) -> compiled runner state


def _build_wide(n_iters=1):
    """Scatter-only program, wide rows. Single engine (gpsimd), single
    SBUF load, one indirect scatter per cache.

    Input blob [2H, WC+1] int32 per core:
      rows 0..H-1  : k update rows (f32 bits), col WC = wide row index
      rows H..2H-1 : v update rows (f32 bits), col WC = wide row index
    Outputs k_out/v_out [WR, WC] int32 — donated cache views.

    n_iters > 1 repeats load+scatter serially (WAR chained) for the
    slope-timing harness; one bass_exec per XLA module is allowed, so
    repetition has to live inside the program.
    """
    import concourse.bass as bass
    import concourse.mybir as mybir

    dt = mybir.dt
    nc = bass.Bass()

    blob = nc.declare_dram_parameter("blob", [2 * H, WC + 1], dt.int32, isOutput=False)
    k_out = nc.declare_dram_parameter("k_out", [WR, WC], dt.int32, isOutput=True)
    v_out = nc.declare_dram_parameter("v_out", [WR, WC], dt.int32, isOutput=True)

    with (
        nc.sbuf_tensor("bt", [2 * H, WC + 1], dt.int32) as bt,
        nc.semaphore("ld_sem") as ld_sem,
        nc.semaphore("sc_sem") as sc_sem,
        nc.Block() as block,
    ):
        @block.gpsimd
        def _(g):
            for i in range(n_iters):
                # WAR: the load overwrites SBUF rows the previous
                # iteration's scatters read from.
                g.wait_ge(sc_sem, 32 * i)
                g.dma_start(out=bt[:, :], in_=blob[:, :]).then_inc(ld_sem, 16)
                g.wait_ge(ld_sem, 16 * (i + 1))
                g.indirect_dma_start(
                    out=k_out[:, :],
                    out_offset=bass.IndirectOffsetOnAxis(ap=bt[0:H, WC : WC + 1], axis=0),
                    in_=bt[0:H, 0:WC],
                    in_offset=None,
                ).then_inc(sc_sem, 16)
                g.indirect_dma_start(
                    out=v_out[:, :],
                    out_offset=bass.IndirectOffsetOnAxis(ap=bt[H : 2 * H, WC : WC + 1], axis=0),
                    in_=bt[H : 2 * H, 0:WC],
                    in_offset=None,
                ).then_inc(sc_sem, 16)
            g.wait_ge(sc_sem, 32 * n_iters)

    return nc


def _build_narrow(n_iters=1):
    """Generic scatter-only fallback: 256 narrow rows per cache."""
    import concourse.bass as bass
    import concourse.mybir as mybir

    dt = mybir.dt
    nc = bass.Bass()

    k_upd = nc.declare_dram_parameter("k_upd", [NROW, D], dt.float32, isOutput=False)
    v_upd = nc.declare_dram_parameter("v_upd", [NROW, D], dt.float32, isOutput=False)
    offsets = nc.declare_dram_parameter("offsets", [NROW, 1], dt.int32, isOutput=False)
    k_out = nc.declare_dram_parameter("k_out", [HS, D], dt.float32, isOutput=True)
    v_out = nc.declare_dram_parameter("v_out", [HS, D], dt.float32, isOutput=True)

    with (
        nc.sbuf_tensor("ku0", [P, D], dt.float32) as ku0,
        nc.sbuf_tensor("ku1", [P, D], dt.float32) as ku1,
        nc.sbuf_tensor("vu0", [P, D], dt.float32) as vu0,
        nc.sbuf_tensor("vu1", [P, D], dt.float32) as vu1,
        nc.sbuf_tensor("off0", [P, 1], dt.int32) as off0,
        nc.sbuf_tensor("off1", [P, 1], dt.int32) as off1,
        nc.semaphore("ld_sem") as ld_sem,
        nc.semaphore("sc_sem") as sc_sem,
        nc.Block() as block,
    ):
        @block.gpsimd
        def _(g):
            loads = [
                (off0[:, :], offsets[0:P, :]),
                (off1[:, :], offsets[P:NROW, :]),
                (ku0[:, :], k_upd[0:P, :]),
                (ku1[:, :], k_upd[P:NROW, :]),
                (vu0[:, :], v_upd[0:P, :]),
                (vu1[:, :], v_upd[P:NROW, :]),
            ]
            scatters = [
                (k_out, off0, ku0),
                (k_out, off1, ku1),
                (v_out, off0, vu0),
                (v_out, off1, vu1),
            ]
            for i in range(n_iters):
                g.wait_ge(sc_sem, 64 * i)
                for dst, src in loads:
                    g.dma_start(out=dst, in_=src).then_inc(ld_sem, 16)
                g.wait_ge(ld_sem, 96 * (i + 1))
                for out_t, off_t, src_t in scatters:
                    g.indirect_dma_start(
                        out=out_t[:, :],
                        out_offset=bass.IndirectOffsetOnAxis(ap=off_t[:, :1], axis=0),
                        in_=src_t[:, :],
                        in_offset=None,
                    ).then_inc(sc_sem, 16)
            g.wait_ge(sc_sem, 64 * n_iters)

    return nc


def get_program(mode, n_iters=1):
    key = (mode, n_iters)
    if key not in _PROGRAMS:
        _PROGRAMS[key] = {"wide": _build_wide, "narrow": _build_narrow}[mode](n_iters)
    return _PROGRAMS[key]


def run_spmd(nc, concat_inputs, concat_inits, n_cores=B, donate=True):
    """Run the bass program on n_cores devices via PJRT (axon).

    concat_inputs: {name: (n_cores*rows, ...) np array} for ExternalInputs.
    concat_inits:  {name: ...} initial contents for ExternalOutputs. When
    donate=True the buffers are donated so the NEFF writes land in them
    in place and unwritten elements keep the init contents.

    Returns list of jax output arrays (concatenated along axis 0).
    """
    import os

    os.environ.setdefault("BASS_NEVER_TRACE", "1")
    import jax
    from jax.sharding import Mesh, PartitionSpec
    from jax.experimental.shard_map import shard_map
    import concourse.mybir as mybir
    from concourse.bass2jax import (
        _bass_exec_p,
        install_neuronx_cc_hook,
        partition_id_tensor,
    )

    key = (id(nc), n_cores, donate)
    state = _JITTED.get(key)
    if state is None:
        install_neuronx_cc_hook()
        partition_name = nc.partition_id_tensor.name if nc.partition_id_tensor else None
        in_names, out_names, out_avals = [], [], []
        for alloc in nc.m.functions[0].allocations:
            if not isinstance(alloc, mybir.MemoryLocationSet):
                continue
            name = alloc.memorylocations[0].name
            if alloc.kind == "ExternalInput":
                if name != partition_name:
                    in_names.append(name)
            elif alloc.kind == "ExternalOutput":
                out_names.append(name)
                out_avals.append(
                    jax.core.ShapedArray(
                        tuple(alloc.tensor_shape), mybir.dt.np(alloc.dtype)
                    )
                )
        n_params = len(in_names)
        all_in = list(in_names) + list(out_names)
        if partition_name is not None:
            all_in.append(partition_name)

        def _body(*args):
            operands = list(args)
            if partition_name is not None:
                operands.append(partition_id_tensor())
            outs = _bass_exec_p.bind(
                *operands,
                out_avals=tuple(out_avals),
                in_names=tuple(all_in),
                out_names=tuple(out_names),
                lowering_input_output_aliases=(),
                sim_require_finite=True,
                sim_require_nnan=True,
                nc=nc,
            )
            return tuple(outs)

        devices = jax.devices()[:n_cores]
        mesh = Mesh(np.asarray(devices), ("core",))
        specs = (PartitionSpec("core"),) * (n_params + len(out_names))
        out_specs = (PartitionSpec("core"),) * len(out_names)
        donate_argnums = (
            tuple(range(n_params, n_params + len(out_names))) if donate else ()
        )
        sharded = jax.jit(
            shard_map(
                _body, mesh=mesh, in_specs=specs, out_specs=out_specs, check_rep=False
            ),
            donate_argnums=donate_argnums,
            keep_unused=True,
        )
        state = (sharded, in_names, out_names)
        _JITTED[key] = state

    sharded, in_names, out_names = state
    args = [concat_inputs[n] for n in in_names] + [concat_inits[n] for n in out_names]
    outs = sharded(*args)
    return dict(zip(out_names, outs))


def _wide_eligible(input_pos):
    pos0 = input_pos[:, 0]
    if np.any(pos0 % T != 0) or np.any(pos0 < 0) or np.any(pos0 + T > S):
        return False
    expect = pos0[:, None] + np.arange(T, dtype=input_pos.dtype)[None, :]
    return bool(np.array_equal(input_pos, expect))


def _pack_wide(input_pos, k, v):
    """Build the concatenated [B*2H, WC+1] int32 input blob."""
    blob = np.empty((B, 2 * H, WC + 1), dtype=np.int32)
    blob[:, 0:H, 0:WC] = k.reshape(B, H, WC).view(np.int32)
    blob[:, H : 2 * H, 0:WC] = v.reshape(B, H, WC).view(np.int32)
    # wide row index of head h for batch b: (h*S + pos0[b]) / T
    pos0 = input_pos[:, 0].astype(np.int64)
    rows = ((np.arange(H, dtype=np.int64)[None, :] * S + pos0[:, None]) // T).astype(
        np.int32
    )  # (B, H)
    blob[:, 0:H, WC] = rows
    blob[:, H : 2 * H, WC] = rows
    return blob.reshape(B * 2 * H, WC + 1)


def kernel(input_pos, k, v, k_cache, v_cache):
    input_pos = np.asarray(input_pos)
    k = np.ascontiguousarray(np.asarray(k, dtype=np.float32))
    v = np.ascontiguousarray(np.asarray(v, dtype=np.float32))
    k_cache = np.ascontiguousarray(np.asarray(k_cache, dtype=np.float32))
    v_cache = np.ascontiguousarray(np.asarray(v_cache, dtype=np.float32))

    if _wide_eligible(input_pos):
        nc = get_program("wide")
        inputs = {"blob": _pack_wide(input_pos, k, v)}
        inits = {
            "k_out": k_cache.view(np.int32).reshape(B * WR, WC),
            "v_out": v_cache.view(np.int32).reshape(B * WR, WC),
        }
        outs = run_spmd(nc, inputs, inits)
        k_out = np.asarray(outs["k_out"]).view(np.float32).reshape(B, H, S, D)
        v_out = np.asarray(outs["v_out"]).view(np.float32).reshape(B, H, S, D)
    else:
        nc = get_program("narrow")
        h_off = np.arange(H, dtype=np.int64)[:, None] * S  # (H, 1)
        offs = (h_off[None] + input_pos[:, None, :].astype(np.int64)).reshape(
            B * NROW, 1
        ).astype(np.int32)
        inputs = {
            "k_upd": k.reshape(B * NROW, D),
            "v_upd": v.reshape(B * NROW, D),
            "offsets": offs,
        }
        inits = {
            "k_out": k_cache.reshape(B * HS, D),
            "v_out": v_cache.reshape(B * HS, D),
        }
        outs = run_spmd(nc, inputs, inits)
        k_out = np.asarray(outs["k_out"]).reshape(B, H, S, D)
        v_out = np.asarray(outs["v_out"]).reshape(B, H, S, D)

    return k_out, v_out


def run_with_results(input_pos, k, v, k_cache, v_cache, trace=False):
    """Compat shim for test.py."""
    return kernel(input_pos, k, v, k_cache, v_cache), None


# revision 5
# speedup vs baseline: 1.0361x; 1.0361x over previous
"""KV-cache scatter kernel for Trainium2 (8 NeuronCores, batch-sharded).

Problem: k_out = k_cache.at[b, :, input_pos[b, t], :].set(k[b, :, t, :])
         (same for v). Shapes: k/v (B,H,T,D)=(8,16,16,128),
         caches (B,H,S,D)=(8,16,4096,128), input_pos (B,T).

Strategy: shard the batch dim across the 8 cores (one batch row each),
and update the caches IN PLACE instead of copying them. The caches are
passed to the device as donated output-init buffers (the jax/PJRT
donation path reuses the staged input buffer as the NEFF output buffer,
so output elements the program never writes retain the cache contents).
The device program is then only the scatter of the update rows.

Two device programs:
- "wide": when each batch row's positions are a contiguous run of T
  aligned to T (the arange case), the cache is viewed as [H*S/T, T*D]
  and each core scatters 16 rows of 8 KiB per cache — one indirect DMA
  per cache, sourced from a single packed SBUF blob.
- "narrow": generic fallback for arbitrary positions — 256 rows of
  512 B per cache via 2+2 indirect DMAs (the baseline scatter scheme).
"""

import numpy as np

B, H, T, D = 8, 16, 16, 128
S = 4096
HS = H * S            # rows in the flattened (H*S, D) cache view
NROW = H * T          # 256 narrow update rows per batch element
P = 128               # SBUF partitions
WR = H * S // T       # 4096 rows in the wide (WR, T*D) cache view
WC = T * D            # 2048 elements per wide row (8 KiB)

_PROGRAMS = {}        # (mode, n_iters) -> bass program
_JITTED = {}          # id(nc) -> compiled runner state


def _build_wide(n_iters=1):
    """Scatter-only program, wide rows. Single engine (gpsimd), single
    SBUF load, one indirect scatter per cache.

    Input blob [2H, WC+1] int32 per core:
      rows 0..H-1  : k update rows (f32 bits), col WC = wide row index
      rows H..2H-1 : v update rows (f32 bits), col WC = wide row index
    Outputs k_out/v_out [WR, WC] int32 — donated cache views.

    n_iters > 1 repeats load+scatter serially (WAR chained) for the
    slope-timing harness; one bass_exec per XLA module is allowed, so
    repetition has to live inside the program.
    """
    import concourse.bass as bass
    import concourse.mybir as mybir

    dt = mybir.dt
    nc = bass.Bass()

    blob = nc.declare_dram_parameter("blob", [2 * H, WC + 1], dt.int32, isOutput=False)
    k_out = nc.declare_dram_parameter("k_out", [WR, WC], dt.int32, isOutput=True)
    v_out = nc.declare_dram_parameter("v_out", [WR, WC], dt.int32, isOutput=True)

    with (
        nc.sbuf_tensor("bt", [2 * H, WC + 1], dt.int32) as bt,
        nc.semaphore("ld_sem") as ld_sem,
        nc.semaphore("sc_sem") as sc_sem,
        nc.Block() as block,
    ):
        @block.gpsimd
        def _(g):
            for i in range(n_iters):
                # WAR: the load overwrites SBUF rows the previous
                # iteration's scatters read from.
                g.wait_ge(sc_sem, 32 * i)
                g.dma_start(out=bt[:, :], in_=blob[:, :]).then_inc(ld_sem, 16)
                g.wait_ge(ld_sem, 16 * (i + 1))
                g.indirect_dma_start(
                    out=k_out[:, :],
                    out_offset=bass.IndirectOffsetOnAxis(ap=bt[0:H, WC : WC + 1], axis=0),
                    in_=bt[0:H, 0:WC],
                    in_offset=None,
                ).then_inc(sc_sem, 16)
                g.indirect_dma_start(
                    out=v_out[:, :],
                    out_offset=bass.IndirectOffsetOnAxis(ap=bt[H : 2 * H, WC : WC + 1], axis=0),
                    in_=bt[H : 2 * H, 0:WC],
                    in_offset=None,
                ).then_inc(sc_sem, 16)
            g.wait_ge(sc_sem, 32 * n_iters)

    return nc


def _build_wide2(n_iters=1):
    """Wide scatter with parallel HWDGE loads: k-blob on sync, v-blob on
    scalar, indirect scatters on gpsimd with per-side semaphores."""
    import concourse.bass as bass
    import concourse.mybir as mybir

    dt = mybir.dt
    nc = bass.Bass()

    kblob = nc.declare_dram_parameter("kblob", [H, WC + 1], dt.int32, isOutput=False)
    vblob = nc.declare_dram_parameter("vblob", [H, WC + 1], dt.int32, isOutput=False)
    k_out = nc.declare_dram_parameter("k_out", [WR, WC], dt.int32, isOutput=True)
    v_out = nc.declare_dram_parameter("v_out", [WR, WC], dt.int32, isOutput=True)

    with (
        nc.sbuf_tensor("kt", [H, WC + 1], dt.int32) as kt,
        nc.sbuf_tensor("vt", [H, WC + 1], dt.int32) as vt,
        nc.semaphore("ldk_sem") as ldk_sem,
        nc.semaphore("ldv_sem") as ldv_sem,
        nc.semaphore("sc_sem") as sc_sem,
        nc.Block() as block,
    ):
        @block.sync
        def _(sync):
            for i in range(n_iters):
                sync.wait_ge(sc_sem, 32 * i)
                sync.dma_start(out=kt[:, :], in_=kblob[:, :]).then_inc(ldk_sem, 16)

        @block.scalar
        def _(scalar):
            for i in range(n_iters):
                scalar.wait_ge(sc_sem, 32 * i)
                scalar.dma_start(out=vt[:, :], in_=vblob[:, :]).then_inc(ldv_sem, 16)

        @block.gpsimd
        def _(g):
            for i in range(n_iters):
                g.wait_ge(ldk_sem, 16 * (i + 1))
                g.indirect_dma_start(
                    out=k_out[:, :],
                    out_offset=bass.IndirectOffsetOnAxis(ap=kt[:, WC : WC + 1], axis=0),
                    in_=kt[:, 0:WC],
                    in_offset=None,
                ).then_inc(sc_sem, 16)
                g.wait_ge(ldv_sem, 16 * (i + 1))
                g.indirect_dma_start(
                    out=v_out[:, :],
                    out_offset=bass.IndirectOffsetOnAxis(ap=vt[:, WC : WC + 1], axis=0),
                    in_=vt[:, 0:WC],
                    in_offset=None,
                ).then_inc(sc_sem, 16)
            g.wait_ge(sc_sem, 32 * n_iters)

    return nc


def _build_narrow(n_iters=1):
    """Generic scatter-only fallback: 256 narrow rows per cache."""
    import concourse.bass as bass
    import concourse.mybir as mybir

    dt = mybir.dt
    nc = bass.Bass()

    k_upd = nc.declare_dram_parameter("k_upd", [NROW, D], dt.float32, isOutput=False)
    v_upd = nc.declare_dram_parameter("v_upd", [NROW, D], dt.float32, isOutput=False)
    offsets = nc.declare_dram_parameter("offsets", [NROW, 1], dt.int32, isOutput=False)
    k_out = nc.declare_dram_parameter("k_out", [HS, D], dt.float32, isOutput=True)
    v_out = nc.declare_dram_parameter("v_out", [HS, D], dt.float32, isOutput=True)

    with (
        nc.sbuf_tensor("ku0", [P, D], dt.float32) as ku0,
        nc.sbuf_tensor("ku1", [P, D], dt.float32) as ku1,
        nc.sbuf_tensor("vu0", [P, D], dt.float32) as vu0,
        nc.sbuf_tensor("vu1", [P, D], dt.float32) as vu1,
        nc.sbuf_tensor("off0", [P, 1], dt.int32) as off0,
        nc.sbuf_tensor("off1", [P, 1], dt.int32) as off1,
        nc.semaphore("ld_sem") as ld_sem,
        nc.semaphore("sc_sem") as sc_sem,
        nc.Block() as block,
    ):
        @block.gpsimd
        def _(g):
            loads = [
                (off0[:, :], offsets[0:P, :]),
                (off1[:, :], offsets[P:NROW, :]),
                (ku0[:, :], k_upd[0:P, :]),
                (ku1[:, :], k_upd[P:NROW, :]),
                (vu0[:, :], v_upd[0:P, :]),
                (vu1[:, :], v_upd[P:NROW, :]),
            ]
            scatters = [
                (k_out, off0, ku0),
                (k_out, off1, ku1),
                (v_out, off0, vu0),
                (v_out, off1, vu1),
            ]
            for i in range(n_iters):
                g.wait_ge(sc_sem, 64 * i)
                for dst, src in loads:
                    g.dma_start(out=dst, in_=src).then_inc(ld_sem, 16)
                g.wait_ge(ld_sem, 96 * (i + 1))
                for out_t, off_t, src_t in scatters:
                    g.indirect_dma_start(
                        out=out_t[:, :],
                        out_offset=bass.IndirectOffsetOnAxis(ap=off_t[:, :1], axis=0),
                        in_=src_t[:, :],
                        in_offset=None,
                    ).then_inc(sc_sem, 16)
            g.wait_ge(sc_sem, 64 * n_iters)

    return nc


def get_program(mode, n_iters=1):
    key = (mode, n_iters)
    if key not in _PROGRAMS:
        _PROGRAMS[key] = {
            "wide": _build_wide,
            "wide2": _build_wide2,
            "narrow": _build_narrow,
        }[mode](n_iters)
    return _PROGRAMS[key]


def run_spmd(nc, concat_inputs, concat_inits, n_cores=B, donate=True):
    """Run the bass program on n_cores devices via PJRT (axon).

    concat_inputs: {name: (n_cores*rows, ...) np array} for ExternalInputs.
    concat_inits:  {name: ...} initial contents for ExternalOutputs. When
    donate=True the buffers are donated so the NEFF writes land in them
    in place and unwritten elements keep the init contents.

    Returns list of jax output arrays (concatenated along axis 0).
    """
    import os

    os.environ.setdefault("BASS_NEVER_TRACE", "1")
    import jax
    from jax.sharding import Mesh, PartitionSpec
    from jax.experimental.shard_map import shard_map
    import concourse.mybir as mybir
    from concourse.bass2jax import (
        _bass_exec_p,
        install_neuronx_cc_hook,
        partition_id_tensor,
    )

    key = (id(nc), n_cores, donate)
    state = _JITTED.get(key)
    if state is None:
        install_neuronx_cc_hook()
        partition_name = nc.partition_id_tensor.name if nc.partition_id_tensor else None
        in_names, out_names, out_avals = [], [], []
        for alloc in nc.m.functions[0].allocations:
            if not isinstance(alloc, mybir.MemoryLocationSet):
                continue
            name = alloc.memorylocations[0].name
            if alloc.kind == "ExternalInput":
                if name != partition_name:
                    in_names.append(name)
            elif alloc.kind == "ExternalOutput":
                out_names.append(name)
                out_avals.append(
                    jax.core.ShapedArray(
                        tuple(alloc.tensor_shape), mybir.dt.np(alloc.dtype)
                    )
                )
        n_params = len(in_names)
        all_in = list(in_names) + list(out_names)
        if partition_name is not None:
            all_in.append(partition_name)

        def _body(*args):
            operands = list(args)
            if partition_name is not None:
                operands.append(partition_id_tensor())
            outs = _bass_exec_p.bind(
                *operands,
                out_avals=tuple(out_avals),
                in_names=tuple(all_in),
                out_names=tuple(out_names),
                lowering_input_output_aliases=(),
                sim_require_finite=True,
                sim_require_nnan=True,
                nc=nc,
            )
            return tuple(outs)

        devices = jax.devices()[:n_cores]
        mesh = Mesh(np.asarray(devices), ("core",))
        specs = (PartitionSpec("core"),) * (n_params + len(out_names))
        out_specs = (PartitionSpec("core"),) * len(out_names)
        donate_argnums = (
            tuple(range(n_params, n_params + len(out_names))) if donate else ()
        )
        sharded = jax.jit(
            shard_map(
                _body, mesh=mesh, in_specs=specs, out_specs=out_specs, check_rep=False
            ),
            donate_argnums=donate_argnums,
            keep_unused=True,
        )
        state = (sharded, in_names, out_names)
        _JITTED[key] = state

    sharded, in_names, out_names = state
    args = [concat_inputs[n] for n in in_names] + [concat_inits[n] for n in out_names]
    outs = sharded(*args)
    return dict(zip(out_names, outs))


def _wide_eligible(input_pos):
    pos0 = input_pos[:, 0]
    if np.any(pos0 % T != 0) or np.any(pos0 < 0) or np.any(pos0 + T > S):
        return False
    expect = pos0[:, None] + np.arange(T, dtype=input_pos.dtype)[None, :]
    return bool(np.array_equal(input_pos, expect))


def _pack_wide(input_pos, k, v):
    """Build the concatenated [B*2H, WC+1] int32 input blob."""
    blob = np.empty((B, 2 * H, WC + 1), dtype=np.int32)
    blob[:, 0:H, 0:WC] = k.reshape(B, H, WC).view(np.int32)
    blob[:, H : 2 * H, 0:WC] = v.reshape(B, H, WC).view(np.int32)
    # wide row index of head h for batch b: (h*S + pos0[b]) / T
    pos0 = input_pos[:, 0].astype(np.int64)
    rows = ((np.arange(H, dtype=np.int64)[None, :] * S + pos0[:, None]) // T).astype(
        np.int32
    )  # (B, H)
    blob[:, 0:H, WC] = rows
    blob[:, H : 2 * H, WC] = rows
    return blob.reshape(B * 2 * H, WC + 1)


def _pack_wide2(input_pos, k, v):
    """Build concatenated [B*H, WC+1] int32 blobs for k and v."""
    pos0 = input_pos[:, 0].astype(np.int64)
    rows = ((np.arange(H, dtype=np.int64)[None, :] * S + pos0[:, None]) // T).astype(
        np.int32
    )  # (B, H)
    kb = np.empty((B, H, WC + 1), dtype=np.int32)
    vb = np.empty((B, H, WC + 1), dtype=np.int32)
    kb[:, :, 0:WC] = k.reshape(B, H, WC).view(np.int32)
    vb[:, :, 0:WC] = v.reshape(B, H, WC).view(np.int32)
    kb[:, :, WC] = rows
    vb[:, :, WC] = rows
    return kb.reshape(B * H, WC + 1), vb.reshape(B * H, WC + 1)


def kernel(input_pos, k, v, k_cache, v_cache):
    input_pos = np.asarray(input_pos)
    k = np.ascontiguousarray(np.asarray(k, dtype=np.float32))
    v = np.ascontiguousarray(np.asarray(v, dtype=np.float32))
    k_cache = np.ascontiguousarray(np.asarray(k_cache, dtype=np.float32))
    v_cache = np.ascontiguousarray(np.asarray(v_cache, dtype=np.float32))

    if _wide_eligible(input_pos):
        nc = get_program("wide")
        inputs = {"blob": _pack_wide(input_pos, k, v)}
        inits = {
            "k_out": k_cache.view(np.int32).reshape(B * WR, WC),
            "v_out": v_cache.view(np.int32).reshape(B * WR, WC),
        }
        outs = run_spmd(nc, inputs, inits)
        k_out = np.asarray(outs["k_out"]).view(np.float32).reshape(B, H, S, D)
        v_out = np.asarray(outs["v_out"]).view(np.float32).reshape(B, H, S, D)
    else:
        nc = get_program("narrow")
        h_off = np.arange(H, dtype=np.int64)[:, None] * S  # (H, 1)
        offs = (h_off[None] + input_pos[:, None, :].astype(np.int64)).reshape(
            B * NROW, 1
        ).astype(np.int32)
        inputs = {
            "k_upd": k.reshape(B * NROW, D),
            "v_upd": v.reshape(B * NROW, D),
            "offsets": offs,
        }
        inits = {
            "k_out": k_cache.reshape(B * HS, D),
            "v_out": v_cache.reshape(B * HS, D),
        }
        outs = run_spmd(nc, inputs, inits)
        k_out = np.asarray(outs["k_out"]).reshape(B, H, S, D)
        v_out = np.asarray(outs["v_out"]).reshape(B, H, S, D)

    return k_out, v_out


def run_with_results(input_pos, k, v, k_cache, v_cache, trace=False):
    """Compat shim for test.py."""
    return kernel(input_pos, k, v, k_cache, v_cache), None


# revision 7
# speedup vs baseline: 2.0490x; 1.9776x over previous
"""KV-cache scatter kernel for Trainium2 (8 NeuronCores, batch-sharded).

Problem: k_out = k_cache.at[b, :, input_pos[b, t], :].set(k[b, :, t, :])
         (same for v). Shapes: k/v (B,H,T,D)=(8,16,16,128),
         caches (B,H,S,D)=(8,16,4096,128), input_pos (B,T).

Strategy: shard the batch dim across the 8 cores (one batch row each),
and update the caches IN PLACE instead of copying them. The caches are
passed to the device as donated output-init buffers (the jax/PJRT
donation path reuses the staged input buffer as the NEFF output buffer,
so output elements the program never writes retain the cache contents).
The device program is then only the scatter of the update rows.

Two device programs:
- "wide": when each batch row's positions are a contiguous run of T
  aligned to T (the arange case), the cache is viewed as [H*S/T, T*D]
  and each core scatters 16 rows of 8 KiB per cache — one indirect DMA
  per cache, sourced from a single packed SBUF blob.
- "narrow": generic fallback for arbitrary positions — 256 rows of
  512 B per cache via 2+2 indirect DMAs (the baseline scatter scheme).
"""

import numpy as np

B, H, T, D = 8, 16, 16, 128
S = 4096
HS = H * S            # rows in the flattened (H*S, D) cache view
NROW = H * T          # 256 narrow update rows per batch element
P = 128               # SBUF partitions
WR = H * S // T       # 4096 rows in the wide (WR, T*D) cache view
WC = T * D            # 2048 elements per wide row (8 KiB)

_PROGRAMS = {}        # (mode, n_iters) -> bass program
_JITTED = {}          # id(nc) -> compiled runner state


def _build_wide(n_iters=1):
    """Scatter-only program, wide rows. Single engine (gpsimd), single
    SBUF load, one indirect scatter per cache.

    Input blob [2H, WC+1] int32 per core:
      rows 0..H-1  : k update rows (f32 bits), col WC = wide row index
      rows H..2H-1 : v update rows (f32 bits), col WC = wide row index
    Outputs k_out/v_out [WR, WC] int32 — donated cache views.

    n_iters > 1 repeats load+scatter serially (WAR chained) for the
    slope-timing harness; one bass_exec per XLA module is allowed, so
    repetition has to live inside the program.
    """
    import concourse.bass as bass
    import concourse.mybir as mybir

    dt = mybir.dt
    nc = bass.Bass()

    blob = nc.declare_dram_parameter("blob", [2 * H, WC + 1], dt.int32, isOutput=False)
    k_out = nc.declare_dram_parameter("k_out", [WR, WC], dt.int32, isOutput=True)
    v_out = nc.declare_dram_parameter("v_out", [WR, WC], dt.int32, isOutput=True)

    with (
        nc.sbuf_tensor("bt", [2 * H, WC + 1], dt.int32) as bt,
        nc.semaphore("ld_sem") as ld_sem,
        nc.semaphore("sc_sem") as sc_sem,
        nc.Block() as block,
    ):
        @block.gpsimd
        def _(g):
            for i in range(n_iters):
                # WAR: the load overwrites SBUF rows the previous
                # iteration's scatters read from.
                g.wait_ge(sc_sem, 32 * i)
                g.dma_start(out=bt[:, :], in_=blob[:, :]).then_inc(ld_sem, 16)
                g.wait_ge(ld_sem, 16 * (i + 1))
                g.indirect_dma_start(
                    out=k_out[:, :],
                    out_offset=bass.IndirectOffsetOnAxis(ap=bt[0:H, WC : WC + 1], axis=0),
                    in_=bt[0:H, 0:WC],
                    in_offset=None,
                ).then_inc(sc_sem, 16)
                g.indirect_dma_start(
                    out=v_out[:, :],
                    out_offset=bass.IndirectOffsetOnAxis(ap=bt[H : 2 * H, WC : WC + 1], axis=0),
                    in_=bt[H : 2 * H, 0:WC],
                    in_offset=None,
                ).then_inc(sc_sem, 16)
            g.wait_ge(sc_sem, 32 * n_iters)

    return nc


def _build_wide2(n_iters=1):
    """Wide scatter with parallel HWDGE loads: k-blob on sync, v-blob on
    scalar, indirect scatters on gpsimd with per-side semaphores."""
    import concourse.bass as bass
    import concourse.mybir as mybir

    dt = mybir.dt
    nc = bass.Bass()

    kblob = nc.declare_dram_parameter("kblob", [H, WC + 1], dt.int32, isOutput=False)
    vblob = nc.declare_dram_parameter("vblob", [H, WC + 1], dt.int32, isOutput=False)
    k_out = nc.declare_dram_parameter("k_out", [WR, WC], dt.int32, isOutput=True)
    v_out = nc.declare_dram_parameter("v_out", [WR, WC], dt.int32, isOutput=True)

    with (
        nc.sbuf_tensor("kt", [H, WC + 1], dt.int32) as kt,
        nc.sbuf_tensor("vt", [H, WC + 1], dt.int32) as vt,
        nc.semaphore("ldk_sem") as ldk_sem,
        nc.semaphore("ldv_sem") as ldv_sem,
        nc.semaphore("sc_sem") as sc_sem,
        nc.Block() as block,
    ):
        @block.sync
        def _(sync):
            for i in range(n_iters):
                sync.wait_ge(sc_sem, 32 * i)
                sync.dma_start(out=kt[:, :], in_=kblob[:, :]).then_inc(ldk_sem, 16)

        @block.scalar
        def _(scalar):
            for i in range(n_iters):
                scalar.wait_ge(sc_sem, 32 * i)
                scalar.dma_start(out=vt[:, :], in_=vblob[:, :]).then_inc(ldv_sem, 16)

        @block.gpsimd
        def _(g):
            for i in range(n_iters):
                g.wait_ge(ldk_sem, 16 * (i + 1))
                g.indirect_dma_start(
                    out=k_out[:, :],
                    out_offset=bass.IndirectOffsetOnAxis(ap=kt[:, WC : WC + 1], axis=0),
                    in_=kt[:, 0:WC],
                    in_offset=None,
                ).then_inc(sc_sem, 16)
                g.wait_ge(ldv_sem, 16 * (i + 1))
                g.indirect_dma_start(
                    out=v_out[:, :],
                    out_offset=bass.IndirectOffsetOnAxis(ap=vt[:, WC : WC + 1], axis=0),
                    in_=vt[:, 0:WC],
                    in_offset=None,
                ).then_inc(sc_sem, 16)
            g.wait_ge(sc_sem, 32 * n_iters)

    return nc


def _build_wide2_loadonly(n_iters=1):
    """Bisect probe: the wide2 load level only, serialized on completion."""
    import concourse.mybir as mybir

    dt = mybir.dt
    import concourse.bass as bass

    nc = bass.Bass()
    kblob = nc.declare_dram_parameter("kblob", [H, WC + 1], dt.int32, isOutput=False)
    vblob = nc.declare_dram_parameter("vblob", [H, WC + 1], dt.int32, isOutput=False)
    k_out = nc.declare_dram_parameter("k_out", [1, WC], dt.int32, isOutput=True)
    v_out = nc.declare_dram_parameter("v_out", [1, WC], dt.int32, isOutput=True)

    with (
        nc.sbuf_tensor("kt", [H, WC + 1], dt.int32) as kt,
        nc.sbuf_tensor("vt", [H, WC + 1], dt.int32) as vt,
        nc.semaphore("ldk_sem") as ldk_sem,
        nc.semaphore("ldv_sem") as ldv_sem,
        nc.Block() as block,
    ):
        @block.sync
        def _(sync):
            for i in range(n_iters):
                sync.wait_ge(ldk_sem, 16 * i)
                sync.dma_start(out=kt[:, :], in_=kblob[:, :]).then_inc(ldk_sem, 16)
            sync.wait_ge(ldk_sem, 16 * n_iters)

        @block.scalar
        def _(scalar):
            for i in range(n_iters):
                scalar.wait_ge(ldv_sem, 16 * i)
                scalar.dma_start(out=vt[:, :], in_=vblob[:, :]).then_inc(ldv_sem, 16)
            scalar.wait_ge(ldv_sem, 16 * n_iters)

        @block.gpsimd
        def _(g):
            g.dma_start(out=k_out[:, :], in_=kt[0:1, 0:WC])
            g.dma_start(out=v_out[:, :], in_=vt[0:1, 0:WC])

    return nc


def _build_wide2_scatteronly(n_iters=1):
    """Bisect probe: load once, then n_iters serialized scatter rounds."""
    import concourse.bass as bass
    import concourse.mybir as mybir

    dt = mybir.dt
    nc = bass.Bass()
    kblob = nc.declare_dram_parameter("kblob", [H, WC + 1], dt.int32, isOutput=False)
    vblob = nc.declare_dram_parameter("vblob", [H, WC + 1], dt.int32, isOutput=False)
    k_out = nc.declare_dram_parameter("k_out", [WR, WC], dt.int32, isOutput=True)
    v_out = nc.declare_dram_parameter("v_out", [WR, WC], dt.int32, isOutput=True)

    with (
        nc.sbuf_tensor("kt", [H, WC + 1], dt.int32) as kt,
        nc.sbuf_tensor("vt", [H, WC + 1], dt.int32) as vt,
        nc.semaphore("ld_sem") as ld_sem,
        nc.semaphore("sc_sem") as sc_sem,
        nc.Block() as block,
    ):
        @block.sync
        def _(sync):
            sync.dma_start(out=kt[:, :], in_=kblob[:, :]).then_inc(ld_sem, 16)
            sync.dma_start(out=vt[:, :], in_=vblob[:, :]).then_inc(ld_sem, 16)

        @block.gpsimd
        def _(g):
            g.wait_ge(ld_sem, 32)
            for i in range(n_iters):
                g.wait_ge(sc_sem, 32 * i)
                g.indirect_dma_start(
                    out=k_out[:, :],
                    out_offset=bass.IndirectOffsetOnAxis(ap=kt[:, WC : WC + 1], axis=0),
                    in_=kt[:, 0:WC],
                    in_offset=None,
                ).then_inc(sc_sem, 16)
                g.indirect_dma_start(
                    out=v_out[:, :],
                    out_offset=bass.IndirectOffsetOnAxis(ap=vt[:, WC : WC + 1], axis=0),
                    in_=vt[:, 0:WC],
                    in_offset=None,
                ).then_inc(sc_sem, 16)
            g.wait_ge(sc_sem, 32 * n_iters)

    return nc


def _build_narrow(n_iters=1):
    """Generic scatter-only fallback: 256 narrow rows per cache."""
    import concourse.bass as bass
    import concourse.mybir as mybir

    dt = mybir.dt
    nc = bass.Bass()

    k_upd = nc.declare_dram_parameter("k_upd", [NROW, D], dt.float32, isOutput=False)
    v_upd = nc.declare_dram_parameter("v_upd", [NROW, D], dt.float32, isOutput=False)
    offsets = nc.declare_dram_parameter("offsets", [NROW, 1], dt.int32, isOutput=False)
    k_out = nc.declare_dram_parameter("k_out", [HS, D], dt.float32, isOutput=True)
    v_out = nc.declare_dram_parameter("v_out", [HS, D], dt.float32, isOutput=True)

    with (
        nc.sbuf_tensor("ku0", [P, D], dt.float32) as ku0,
        nc.sbuf_tensor("ku1", [P, D], dt.float32) as ku1,
        nc.sbuf_tensor("vu0", [P, D], dt.float32) as vu0,
        nc.sbuf_tensor("vu1", [P, D], dt.float32) as vu1,
        nc.sbuf_tensor("off0", [P, 1], dt.int32) as off0,
        nc.sbuf_tensor("off1", [P, 1], dt.int32) as off1,
        nc.semaphore("ld_sem") as ld_sem,
        nc.semaphore("sc_sem") as sc_sem,
        nc.Block() as block,
    ):
        @block.gpsimd
        def _(g):
            loads = [
                (off0[:, :], offsets[0:P, :]),
                (off1[:, :], offsets[P:NROW, :]),
                (ku0[:, :], k_upd[0:P, :]),
                (ku1[:, :], k_upd[P:NROW, :]),
                (vu0[:, :], v_upd[0:P, :]),
                (vu1[:, :], v_upd[P:NROW, :]),
            ]
            scatters = [
                (k_out, off0, ku0),
                (k_out, off1, ku1),
                (v_out, off0, vu0),
                (v_out, off1, vu1),
            ]
            for i in range(n_iters):
                g.wait_ge(sc_sem, 64 * i)
                for dst, src in loads:
                    g.dma_start(out=dst, in_=src).then_inc(ld_sem, 16)
                g.wait_ge(ld_sem, 96 * (i + 1))
                for out_t, off_t, src_t in scatters:
                    g.indirect_dma_start(
                        out=out_t[:, :],
                        out_offset=bass.IndirectOffsetOnAxis(ap=off_t[:, :1], axis=0),
                        in_=src_t[:, :],
                        in_offset=None,
                    ).then_inc(sc_sem, 16)
            g.wait_ge(sc_sem, 64 * n_iters)

    return nc


def get_program(mode, n_iters=1):
    key = (mode, n_iters)
    if key not in _PROGRAMS:
        _PROGRAMS[key] = {
            "wide": _build_wide,
            "wide2": _build_wide2,
            "wide2_loadonly": _build_wide2_loadonly,
            "wide2_scatteronly": _build_wide2_scatteronly,
            "narrow": _build_narrow,
        }[mode](n_iters)
    return _PROGRAMS[key]


def run_spmd(nc, concat_inputs, concat_inits, n_cores=B, donate=True):
    """Run the bass program on n_cores devices via PJRT (axon).

    concat_inputs: {name: (n_cores*rows, ...) np array} for ExternalInputs.
    concat_inits:  {name: ...} initial contents for ExternalOutputs. When
    donate=True the buffers are donated so the NEFF writes land in them
    in place and unwritten elements keep the init contents.

    Returns list of jax output arrays (concatenated along axis 0).
    """
    import os

    os.environ.setdefault("BASS_NEVER_TRACE", "1")
    import jax
    from jax.sharding import Mesh, PartitionSpec
    from jax.experimental.shard_map import shard_map
    import concourse.mybir as mybir
    from concourse.bass2jax import (
        _bass_exec_p,
        install_neuronx_cc_hook,
        partition_id_tensor,
    )

    key = (id(nc), n_cores, donate)
    state = _JITTED.get(key)
    if state is None:
        install_neuronx_cc_hook()
        partition_name = nc.partition_id_tensor.name if nc.partition_id_tensor else None
        in_names, out_names, out_avals = [], [], []
        for alloc in nc.m.functions[0].allocations:
            if not isinstance(alloc, mybir.MemoryLocationSet):
                continue
            name = alloc.memorylocations[0].name
            if alloc.kind == "ExternalInput":
                if name != partition_name:
                    in_names.append(name)
            elif alloc.kind == "ExternalOutput":
                out_names.append(name)
                out_avals.append(
                    jax.core.ShapedArray(
                        tuple(alloc.tensor_shape), mybir.dt.np(alloc.dtype)
                    )
                )
        n_params = len(in_names)
        all_in = list(in_names) + list(out_names)
        if partition_name is not None:
            all_in.append(partition_name)

        def _body(*args):
            operands = list(args)
            if partition_name is not None:
                operands.append(partition_id_tensor())
            outs = _bass_exec_p.bind(
                *operands,
                out_avals=tuple(out_avals),
                in_names=tuple(all_in),
                out_names=tuple(out_names),
                lowering_input_output_aliases=(),
                sim_require_finite=True,
                sim_require_nnan=True,
                nc=nc,
            )
            return tuple(outs)

        devices = jax.devices()[:n_cores]
        mesh = Mesh(np.asarray(devices), ("core",))
        specs = (PartitionSpec("core"),) * (n_params + len(out_names))
        out_specs = (PartitionSpec("core"),) * len(out_names)
        donate_argnums = (
            tuple(range(n_params, n_params + len(out_names))) if donate else ()
        )
        sharded = jax.jit(
            shard_map(
                _body, mesh=mesh, in_specs=specs, out_specs=out_specs, check_rep=False
            ),
            donate_argnums=donate_argnums,
            keep_unused=True,
        )
        state = (sharded, in_names, out_names)
        _JITTED[key] = state

    sharded, in_names, out_names = state
    args = [concat_inputs[n] for n in in_names] + [concat_inits[n] for n in out_names]
    outs = sharded(*args)
    return dict(zip(out_names, outs))


def _wide_eligible(input_pos):
    pos0 = input_pos[:, 0]
    if np.any(pos0 % T != 0) or np.any(pos0 < 0) or np.any(pos0 + T > S):
        return False
    expect = pos0[:, None] + np.arange(T, dtype=input_pos.dtype)[None, :]
    return bool(np.array_equal(input_pos, expect))


def _pack_wide(input_pos, k, v):
    """Build the concatenated [B*2H, WC+1] int32 input blob."""
    blob = np.empty((B, 2 * H, WC + 1), dtype=np.int32)
    blob[:, 0:H, 0:WC] = k.reshape(B, H, WC).view(np.int32)
    blob[:, H : 2 * H, 0:WC] = v.reshape(B, H, WC).view(np.int32)
    # wide row index of head h for batch b: (h*S + pos0[b]) / T
    pos0 = input_pos[:, 0].astype(np.int64)
    rows = ((np.arange(H, dtype=np.int64)[None, :] * S + pos0[:, None]) // T).astype(
        np.int32
    )  # (B, H)
    blob[:, 0:H, WC] = rows
    blob[:, H : 2 * H, WC] = rows
    return blob.reshape(B * 2 * H, WC + 1)


def _pack_wide2(input_pos, k, v):
    """Build concatenated [B*H, WC+1] int32 blobs for k and v."""
    pos0 = input_pos[:, 0].astype(np.int64)
    rows = ((np.arange(H, dtype=np.int64)[None, :] * S + pos0[:, None]) // T).astype(
        np.int32
    )  # (B, H)
    kb = np.empty((B, H, WC + 1), dtype=np.int32)
    vb = np.empty((B, H, WC + 1), dtype=np.int32)
    kb[:, :, 0:WC] = k.reshape(B, H, WC).view(np.int32)
    vb[:, :, 0:WC] = v.reshape(B, H, WC).view(np.int32)
    kb[:, :, WC] = rows
    vb[:, :, WC] = rows
    return kb.reshape(B * H, WC + 1), vb.reshape(B * H, WC + 1)


def kernel(input_pos, k, v, k_cache, v_cache):
    input_pos = np.asarray(input_pos)
    k = np.ascontiguousarray(np.asarray(k, dtype=np.float32))
    v = np.ascontiguousarray(np.asarray(v, dtype=np.float32))
    k_cache = np.ascontiguousarray(np.asarray(k_cache, dtype=np.float32))
    v_cache = np.ascontiguousarray(np.asarray(v_cache, dtype=np.float32))

    if _wide_eligible(input_pos):
        nc = get_program("wide")
        inputs = {"blob": _pack_wide(input_pos, k, v)}
        inits = {
            "k_out": k_cache.view(np.int32).reshape(B * WR, WC),
            "v_out": v_cache.view(np.int32).reshape(B * WR, WC),
        }
        outs = run_spmd(nc, inputs, inits)
        k_out = np.asarray(outs["k_out"]).view(np.float32).reshape(B, H, S, D)
        v_out = np.asarray(outs["v_out"]).view(np.float32).reshape(B, H, S, D)
    else:
        nc = get_program("narrow")
        h_off = np.arange(H, dtype=np.int64)[:, None] * S  # (H, 1)
        offs = (h_off[None] + input_pos[:, None, :].astype(np.int64)).reshape(
            B * NROW, 1
        ).astype(np.int32)
        inputs = {
            "k_upd": k.reshape(B * NROW, D),
            "v_upd": v.reshape(B * NROW, D),
            "offsets": offs,
        }
        inits = {
            "k_out": k_cache.reshape(B * HS, D),
            "v_out": v_cache.reshape(B * HS, D),
        }
        outs = run_spmd(nc, inputs, inits)
        k_out = np.asarray(outs["k_out"]).reshape(B, H, S, D)
        v_out = np.asarray(outs["v_out"]).reshape(B, H, S, D)

    return k_out, v_out


def run_with_results(input_pos, k, v, k_cache, v_cache, trace=False):
    """Compat shim for test.py."""
    return kernel(input_pos, k, v, k_cache, v_cache), None


# revision 9
# speedup vs baseline: 3.9139x; 1.9101x over previous
"""KV-cache scatter kernel for Trainium2 (8 NeuronCores, batch-sharded).

Problem: k_out = k_cache.at[b, :, input_pos[b, t], :].set(k[b, :, t, :])
         (same for v). Shapes: k/v (B,H,T,D)=(8,16,16,128),
         caches (B,H,S,D)=(8,16,4096,128), input_pos (B,T).

Strategy: shard the batch dim across the 8 cores (one batch row each),
and update the caches IN PLACE instead of copying them. The caches are
passed to the device as donated output-init buffers (the jax/PJRT
donation path reuses the staged input buffer as the NEFF output buffer,
so output elements the program never writes retain the cache contents).
The device program is then only the scatter of the update rows.

Two device programs:
- "wide": when each batch row's positions are a contiguous run of T
  aligned to T (the arange case), the cache is viewed as [H*S/T, T*D]
  and each core scatters 16 rows of 8 KiB per cache — one indirect DMA
  per cache, sourced from a single packed SBUF blob.
- "narrow": generic fallback for arbitrary positions — 256 rows of
  512 B per cache via 2+2 indirect DMAs (the baseline scatter scheme).
"""

import numpy as np

B, H, T, D = 8, 16, 16, 128
S = 4096
HS = H * S            # rows in the flattened (H*S, D) cache view
NROW = H * T          # 256 narrow update rows per batch element
P = 128               # SBUF partitions
WR = H * S // T       # 4096 rows in the wide (WR, T*D) cache view
WC = T * D            # 2048 elements per wide row (8 KiB)

_PROGRAMS = {}        # (mode, n_iters) -> bass program
_JITTED = {}          # id(nc) -> compiled runner state


def _build_wide(n_iters=1):
    """Scatter-only program, wide rows. Single engine (gpsimd), single
    SBUF load, one indirect scatter per cache.

    Input blob [2H, WC+1] int32 per core:
      rows 0..H-1  : k update rows (f32 bits), col WC = wide row index
      rows H..2H-1 : v update rows (f32 bits), col WC = wide row index
    Outputs k_out/v_out [WR, WC] int32 — donated cache views.

    n_iters > 1 repeats load+scatter serially (WAR chained) for the
    slope-timing harness; one bass_exec per XLA module is allowed, so
    repetition has to live inside the program.
    """
    import concourse.bass as bass
    import concourse.mybir as mybir

    dt = mybir.dt
    nc = bass.Bass()

    blob = nc.declare_dram_parameter("blob", [2 * H, WC + 1], dt.int32, isOutput=False)
    k_out = nc.declare_dram_parameter("k_out", [WR, WC], dt.int32, isOutput=True)
    v_out = nc.declare_dram_parameter("v_out", [WR, WC], dt.int32, isOutput=True)

    with (
        nc.sbuf_tensor("bt", [2 * H, WC + 1], dt.int32) as bt,
        nc.semaphore("ld_sem") as ld_sem,
        nc.semaphore("sc_sem") as sc_sem,
        nc.Block() as block,
    ):
        @block.gpsimd
        def _(g):
            for i in range(n_iters):
                # WAR: the load overwrites SBUF rows the previous
                # iteration's scatters read from.
                g.wait_ge(sc_sem, 32 * i)
                g.dma_start(out=bt[:, :], in_=blob[:, :]).then_inc(ld_sem, 16)
                g.wait_ge(ld_sem, 16 * (i + 1))
                g.indirect_dma_start(
                    out=k_out[:, :],
                    out_offset=bass.IndirectOffsetOnAxis(ap=bt[0:H, WC : WC + 1], axis=0),
                    in_=bt[0:H, 0:WC],
                    in_offset=None,
                ).then_inc(sc_sem, 16)
                g.indirect_dma_start(
                    out=v_out[:, :],
                    out_offset=bass.IndirectOffsetOnAxis(ap=bt[H : 2 * H, WC : WC + 1], axis=0),
                    in_=bt[H : 2 * H, 0:WC],
                    in_offset=None,
                ).then_inc(sc_sem, 16)
            g.wait_ge(sc_sem, 32 * n_iters)

    return nc


def _build_wide2(n_iters=1):
    """Wide scatter with parallel HWDGE loads: k-blob on sync, v-blob on
    scalar, indirect scatters on gpsimd with per-side semaphores."""
    import concourse.bass as bass
    import concourse.mybir as mybir

    dt = mybir.dt
    nc = bass.Bass()

    kblob = nc.declare_dram_parameter("kblob", [H, WC + 1], dt.int32, isOutput=False)
    vblob = nc.declare_dram_parameter("vblob", [H, WC + 1], dt.int32, isOutput=False)
    k_out = nc.declare_dram_parameter("k_out", [WR, WC], dt.int32, isOutput=True)
    v_out = nc.declare_dram_parameter("v_out", [WR, WC], dt.int32, isOutput=True)

    with (
        nc.sbuf_tensor("kt", [H, WC + 1], dt.int32) as kt,
        nc.sbuf_tensor("vt", [H, WC + 1], dt.int32) as vt,
        nc.semaphore("ldk_sem") as ldk_sem,
        nc.semaphore("ldv_sem") as ldv_sem,
        nc.semaphore("sc_sem") as sc_sem,
        nc.Block() as block,
    ):
        @block.sync
        def _(sync):
            for i in range(n_iters):
                sync.wait_ge(sc_sem, 32 * i)
                sync.dma_start(out=kt[:, :], in_=kblob[:, :]).then_inc(ldk_sem, 16)

        @block.scalar
        def _(scalar):
            for i in range(n_iters):
                scalar.wait_ge(sc_sem, 32 * i)
                scalar.dma_start(out=vt[:, :], in_=vblob[:, :]).then_inc(ldv_sem, 16)

        @block.gpsimd
        def _(g):
            for i in range(n_iters):
                g.wait_ge(ldk_sem, 16 * (i + 1))
                g.indirect_dma_start(
                    out=k_out[:, :],
                    out_offset=bass.IndirectOffsetOnAxis(ap=kt[:, WC : WC + 1], axis=0),
                    in_=kt[:, 0:WC],
                    in_offset=None,
                ).then_inc(sc_sem, 16)
                g.wait_ge(ldv_sem, 16 * (i + 1))
                g.indirect_dma_start(
                    out=v_out[:, :],
                    out_offset=bass.IndirectOffsetOnAxis(ap=vt[:, WC : WC + 1], axis=0),
                    in_=vt[:, 0:WC],
                    in_offset=None,
                ).then_inc(sc_sem, 16)
            g.wait_ge(sc_sem, 32 * n_iters)

    return nc


def _build_wide2_loadonly(n_iters=1):
    """Bisect probe: the wide2 load level only, serialized on completion."""
    import concourse.mybir as mybir

    dt = mybir.dt
    import concourse.bass as bass

    nc = bass.Bass()
    kblob = nc.declare_dram_parameter("kblob", [H, WC + 1], dt.int32, isOutput=False)
    vblob = nc.declare_dram_parameter("vblob", [H, WC + 1], dt.int32, isOutput=False)
    k_out = nc.declare_dram_parameter("k_out", [1, WC], dt.int32, isOutput=True)
    v_out = nc.declare_dram_parameter("v_out", [1, WC], dt.int32, isOutput=True)

    with (
        nc.sbuf_tensor("kt", [H, WC + 1], dt.int32) as kt,
        nc.sbuf_tensor("vt", [H, WC + 1], dt.int32) as vt,
        nc.semaphore("ldk_sem") as ldk_sem,
        nc.semaphore("ldv_sem") as ldv_sem,
        nc.Block() as block,
    ):
        @block.sync
        def _(sync):
            for i in range(n_iters):
                sync.wait_ge(ldk_sem, 16 * i)
                sync.dma_start(out=kt[:, :], in_=kblob[:, :]).then_inc(ldk_sem, 16)
            sync.wait_ge(ldk_sem, 16 * n_iters)

        @block.scalar
        def _(scalar):
            for i in range(n_iters):
                scalar.wait_ge(ldv_sem, 16 * i)
                scalar.dma_start(out=vt[:, :], in_=vblob[:, :]).then_inc(ldv_sem, 16)
            scalar.wait_ge(ldv_sem, 16 * n_iters)

        @block.gpsimd
        def _(g):
            g.dma_start(out=k_out[:, :], in_=kt[0:1, 0:WC])
            g.dma_start(out=v_out[:, :], in_=vt[0:1, 0:WC])

    return nc


def _build_wide2_scatteronly(n_iters=1):
    """Bisect probe: load once, then n_iters serialized scatter rounds."""
    import concourse.bass as bass
    import concourse.mybir as mybir

    dt = mybir.dt
    nc = bass.Bass()
    kblob = nc.declare_dram_parameter("kblob", [H, WC + 1], dt.int32, isOutput=False)
    vblob = nc.declare_dram_parameter("vblob", [H, WC + 1], dt.int32, isOutput=False)
    k_out = nc.declare_dram_parameter("k_out", [WR, WC], dt.int32, isOutput=True)
    v_out = nc.declare_dram_parameter("v_out", [WR, WC], dt.int32, isOutput=True)

    with (
        nc.sbuf_tensor("kt", [H, WC + 1], dt.int32) as kt,
        nc.sbuf_tensor("vt", [H, WC + 1], dt.int32) as vt,
        nc.semaphore("ld_sem") as ld_sem,
        nc.semaphore("sc_sem") as sc_sem,
        nc.Block() as block,
    ):
        @block.sync
        def _(sync):
            sync.dma_start(out=kt[:, :], in_=kblob[:, :]).then_inc(ld_sem, 16)
            sync.dma_start(out=vt[:, :], in_=vblob[:, :]).then_inc(ld_sem, 16)

        @block.gpsimd
        def _(g):
            g.wait_ge(ld_sem, 32)
            for i in range(n_iters):
                g.wait_ge(sc_sem, 32 * i)
                g.indirect_dma_start(
                    out=k_out[:, :],
                    out_offset=bass.IndirectOffsetOnAxis(ap=kt[:, WC : WC + 1], axis=0),
                    in_=kt[:, 0:WC],
                    in_offset=None,
                ).then_inc(sc_sem, 16)
                g.indirect_dma_start(
                    out=v_out[:, :],
                    out_offset=bass.IndirectOffsetOnAxis(ap=vt[:, WC : WC + 1], axis=0),
                    in_=vt[:, 0:WC],
                    in_offset=None,
                ).then_inc(sc_sem, 16)
            g.wait_ge(sc_sem, 32 * n_iters)

    return nc


def _build_switch(pos0s, n_iters=1):
    """Value-specialized scatter: each batch row's positions are a
    contiguous run [pos0, pos0+T), so per core the update is a single
    static strided DRAM->DRAM DMA into the donated cache. The 8 per-core
    base offsets are baked into an 8-way Switch on the partition id —
    no SBUF staging, no indirect DMA. k runs on sync, v on scalar.

    For n_iters > 1 the DMA repeats inside the selected branch,
    serialized on its completion semaphore (slope-timing harness).
    """
    import concourse.bass as bass
    import concourse.mybir as mybir

    dt = mybir.dt
    nc = bass.Bass()

    k_upd = nc.declare_dram_parameter("k_upd", [H, T, D], dt.float32, isOutput=False)
    v_upd = nc.declare_dram_parameter("v_upd", [H, T, D], dt.float32, isOutput=False)
    k_out = nc.declare_dram_parameter("k_out", [H, S, D], dt.float32, isOutput=True)
    v_out = nc.declare_dram_parameter("v_out", [H, S, D], dt.float32, isOutput=True)

    with (
        nc.semaphore("ksc_sem") as ksc_sem,
        nc.semaphore("vsc_sem") as vsc_sem,
        nc.Block() as block,
    ):
        @block.sync
        def _(sync):
            pid = sync.partition_id()
            for b in sync.Switch(pid, B):
                p0 = int(pos0s[b])
                for i in range(n_iters):
                    sync.wait_ge(ksc_sem, 16 * i)
                    sync.dma_start(
                        out=k_out[:, p0 : p0 + T, :], in_=k_upd[:, :, :]
                    ).then_inc(ksc_sem, 16)
            sync.wait_ge(ksc_sem, 16 * n_iters)

        @block.scalar
        def _(scalar):
            pid = scalar.partition_id()
            for b in scalar.Switch(pid, B):
                p0 = int(pos0s[b])
                for i in range(n_iters):
                    scalar.wait_ge(vsc_sem, 16 * i)
                    scalar.dma_start(
                        out=v_out[:, p0 : p0 + T, :], in_=v_upd[:, :, :]
                    ).then_inc(vsc_sem, 16)
            scalar.wait_ge(vsc_sem, 16 * n_iters)

    return nc


def get_switch_program(pos0s, n_iters=1):
    key = ("switch", tuple(int(p) for p in pos0s), n_iters)
    if key not in _PROGRAMS:
        _PROGRAMS[key] = _build_switch(pos0s, n_iters)
    return _PROGRAMS[key]


def _switch_eligible(input_pos):
    """Each row must be a contiguous in-bounds run of T (no alignment
    requirement, unlike the wide paths)."""
    if input_pos.shape != (B, T):
        return False
    pos0 = input_pos[:, 0]
    if np.any(pos0 < 0) or np.any(pos0.astype(np.int64) + T > S):
        return False
    expect = pos0[:, None] + np.arange(T, dtype=input_pos.dtype)[None, :]
    return bool(np.array_equal(input_pos, expect))


def _build_narrow(n_iters=1):
    """Generic scatter-only fallback: 256 narrow rows per cache."""
    import concourse.bass as bass
    import concourse.mybir as mybir

    dt = mybir.dt
    nc = bass.Bass()

    k_upd = nc.declare_dram_parameter("k_upd", [NROW, D], dt.float32, isOutput=False)
    v_upd = nc.declare_dram_parameter("v_upd", [NROW, D], dt.float32, isOutput=False)
    offsets = nc.declare_dram_parameter("offsets", [NROW, 1], dt.int32, isOutput=False)
    k_out = nc.declare_dram_parameter("k_out", [HS, D], dt.float32, isOutput=True)
    v_out = nc.declare_dram_parameter("v_out", [HS, D], dt.float32, isOutput=True)

    with (
        nc.sbuf_tensor("ku0", [P, D], dt.float32) as ku0,
        nc.sbuf_tensor("ku1", [P, D], dt.float32) as ku1,
        nc.sbuf_tensor("vu0", [P, D], dt.float32) as vu0,
        nc.sbuf_tensor("vu1", [P, D], dt.float32) as vu1,
        nc.sbuf_tensor("off0", [P, 1], dt.int32) as off0,
        nc.sbuf_tensor("off1", [P, 1], dt.int32) as off1,
        nc.semaphore("ld_sem") as ld_sem,
        nc.semaphore("sc_sem") as sc_sem,
        nc.Block() as block,
    ):
        @block.gpsimd
        def _(g):
            loads = [
                (off0[:, :], offsets[0:P, :]),
                (off1[:, :], offsets[P:NROW, :]),
                (ku0[:, :], k_upd[0:P, :]),
                (ku1[:, :], k_upd[P:NROW, :]),
                (vu0[:, :], v_upd[0:P, :]),
                (vu1[:, :], v_upd[P:NROW, :]),
            ]
            scatters = [
                (k_out, off0, ku0),
                (k_out, off1, ku1),
                (v_out, off0, vu0),
                (v_out, off1, vu1),
            ]
            for i in range(n_iters):
                g.wait_ge(sc_sem, 64 * i)
                for dst, src in loads:
                    g.dma_start(out=dst, in_=src).then_inc(ld_sem, 16)
                g.wait_ge(ld_sem, 96 * (i + 1))
                for out_t, off_t, src_t in scatters:
                    g.indirect_dma_start(
                        out=out_t[:, :],
                        out_offset=bass.IndirectOffsetOnAxis(ap=off_t[:, :1], axis=0),
                        in_=src_t[:, :],
                        in_offset=None,
                    ).then_inc(sc_sem, 16)
            g.wait_ge(sc_sem, 64 * n_iters)

    return nc


def get_program(mode, n_iters=1):
    key = (mode, n_iters)
    if key not in _PROGRAMS:
        _PROGRAMS[key] = {
            "wide": _build_wide,
            "wide2": _build_wide2,
            "wide2_loadonly": _build_wide2_loadonly,
            "wide2_scatteronly": _build_wide2_scatteronly,
            "narrow": _build_narrow,
        }[mode](n_iters)
    return _PROGRAMS[key]


def run_spmd(nc, concat_inputs, concat_inits, n_cores=B, donate=True):
    """Run the bass program on n_cores devices via PJRT (axon).

    concat_inputs: {name: (n_cores*rows, ...) np array} for ExternalInputs.
    concat_inits:  {name: ...} initial contents for ExternalOutputs. When
    donate=True the buffers are donated so the NEFF writes land in them
    in place and unwritten elements keep the init contents.

    Returns list of jax output arrays (concatenated along axis 0).
    """
    import os

    os.environ.setdefault("BASS_NEVER_TRACE", "1")
    import jax
    from jax.sharding import Mesh, PartitionSpec
    from jax.experimental.shard_map import shard_map
    import concourse.mybir as mybir
    from concourse.bass2jax import (
        _bass_exec_p,
        install_neuronx_cc_hook,
        partition_id_tensor,
    )

    key = (id(nc), n_cores, donate)
    state = _JITTED.get(key)
    if state is None:
        install_neuronx_cc_hook()
        partition_name = nc.partition_id_tensor.name if nc.partition_id_tensor else None
        in_names, out_names, out_avals = [], [], []
        for alloc in nc.m.functions[0].allocations:
            if not isinstance(alloc, mybir.MemoryLocationSet):
                continue
            name = alloc.memorylocations[0].name
            if alloc.kind == "ExternalInput":
                if name != partition_name:
                    in_names.append(name)
            elif alloc.kind == "ExternalOutput":
                out_names.append(name)
                out_avals.append(
                    jax.core.ShapedArray(
                        tuple(alloc.tensor_shape), mybir.dt.np(alloc.dtype)
                    )
                )
        n_params = len(in_names)
        all_in = list(in_names) + list(out_names)
        if partition_name is not None:
            all_in.append(partition_name)

        def _body(*args):
            operands = list(args)
            if partition_name is not None:
                operands.append(partition_id_tensor())
            outs = _bass_exec_p.bind(
                *operands,
                out_avals=tuple(out_avals),
                in_names=tuple(all_in),
                out_names=tuple(out_names),
                lowering_input_output_aliases=(),
                sim_require_finite=True,
                sim_require_nnan=True,
                nc=nc,
            )
            return tuple(outs)

        devices = jax.devices()[:n_cores]
        mesh = Mesh(np.asarray(devices), ("core",))
        specs = (PartitionSpec("core"),) * (n_params + len(out_names))
        out_specs = (PartitionSpec("core"),) * len(out_names)
        donate_argnums = (
            tuple(range(n_params, n_params + len(out_names))) if donate else ()
        )
        sharded = jax.jit(
            shard_map(
                _body, mesh=mesh, in_specs=specs, out_specs=out_specs, check_rep=False
            ),
            donate_argnums=donate_argnums,
            keep_unused=True,
        )
        state = (sharded, in_names, out_names)
        _JITTED[key] = state

    sharded, in_names, out_names = state
    args = [concat_inputs[n] for n in in_names] + [concat_inits[n] for n in out_names]
    outs = sharded(*args)
    return dict(zip(out_names, outs))


def _wide_eligible(input_pos):
    pos0 = input_pos[:, 0]
    if np.any(pos0 % T != 0) or np.any(pos0 < 0) or np.any(pos0 + T > S):
        return False
    expect = pos0[:, None] + np.arange(T, dtype=input_pos.dtype)[None, :]
    return bool(np.array_equal(input_pos, expect))


def _pack_wide(input_pos, k, v):
    """Build the concatenated [B*2H, WC+1] int32 input blob."""
    blob = np.empty((B, 2 * H, WC + 1), dtype=np.int32)
    blob[:, 0:H, 0:WC] = k.reshape(B, H, WC).view(np.int32)
    blob[:, H : 2 * H, 0:WC] = v.reshape(B, H, WC).view(np.int32)
    # wide row index of head h for batch b: (h*S + pos0[b]) / T
    pos0 = input_pos[:, 0].astype(np.int64)
    rows = ((np.arange(H, dtype=np.int64)[None, :] * S + pos0[:, None]) // T).astype(
        np.int32
    )  # (B, H)
    blob[:, 0:H, WC] = rows
    blob[:, H : 2 * H, WC] = rows
    return blob.reshape(B * 2 * H, WC + 1)


def _pack_wide2(input_pos, k, v):
    """Build concatenated [B*H, WC+1] int32 blobs for k and v."""
    pos0 = input_pos[:, 0].astype(np.int64)
    rows = ((np.arange(H, dtype=np.int64)[None, :] * S + pos0[:, None]) // T).astype(
        np.int32
    )  # (B, H)
    kb = np.empty((B, H, WC + 1), dtype=np.int32)
    vb = np.empty((B, H, WC + 1), dtype=np.int32)
    kb[:, :, 0:WC] = k.reshape(B, H, WC).view(np.int32)
    vb[:, :, 0:WC] = v.reshape(B, H, WC).view(np.int32)
    kb[:, :, WC] = rows
    vb[:, :, WC] = rows
    return kb.reshape(B * H, WC + 1), vb.reshape(B * H, WC + 1)


def kernel(input_pos, k, v, k_cache, v_cache):
    input_pos = np.asarray(input_pos)
    k = np.ascontiguousarray(np.asarray(k, dtype=np.float32))
    v = np.ascontiguousarray(np.asarray(v, dtype=np.float32))
    k_cache = np.ascontiguousarray(np.asarray(k_cache, dtype=np.float32))
    v_cache = np.ascontiguousarray(np.asarray(v_cache, dtype=np.float32))

    if _switch_eligible(input_pos):
        nc = get_switch_program(input_pos[:, 0])
        inputs = {
            "k_upd": k.reshape(B * H, T, D),
            "v_upd": v.reshape(B * H, T, D),
        }
        inits = {
            "k_out": k_cache.reshape(B * H, S, D),
            "v_out": v_cache.reshape(B * H, S, D),
        }
        outs = run_spmd(nc, inputs, inits)
        k_out = np.asarray(outs["k_out"]).reshape(B, H, S, D)
        v_out = np.asarray(outs["v_out"]).reshape(B, H, S, D)
    elif _wide_eligible(input_pos):
        nc = get_program("wide")
        inputs = {"blob": _pack_wide(input_pos, k, v)}
        inits = {
            "k_out": k_cache.view(np.int32).reshape(B * WR, WC),
            "v_out": v_cache.view(np.int32).reshape(B * WR, WC),
        }
        outs = run_spmd(nc, inputs, inits)
        k_out = np.asarray(outs["k_out"]).view(np.float32).reshape(B, H, S, D)
        v_out = np.asarray(outs["v_out"]).view(np.float32).reshape(B, H, S, D)
    else:
        nc = get_program("narrow")
        h_off = np.arange(H, dtype=np.int64)[:, None] * S  # (H, 1)
        offs = (h_off[None] + input_pos[:, None, :].astype(np.int64)).reshape(
            B * NROW, 1
        ).astype(np.int32)
        inputs = {
            "k_upd": k.reshape(B * NROW, D),
            "v_upd": v.reshape(B * NROW, D),
            "offsets": offs,
        }
        inits = {
            "k_out": k_cache.reshape(B * HS, D),
            "v_out": v_cache.reshape(B * HS, D),
        }
        outs = run_spmd(nc, inputs, inits)
        k_out = np.asarray(outs["k_out"]).reshape(B, H, S, D)
        v_out = np.asarray(outs["v_out"]).reshape(B, H, S, D)

    return k_out, v_out


def run_with_results(input_pos, k, v, k_cache, v_cache, trace=False):
    """Compat shim for test.py."""
    return kernel(input_pos, k, v, k_cache, v_cache), None
